# revision 26
# baseline (speedup 1.0000x reference)
"""GCN 2-layer message-passing block on 8 Trainium2 NeuronCores — v4.

Collapsed algebra: z = (S^2 x) W12^T + c2 vv^T + cvec b2^T with
S = D^-1/2 A D^-1/2, W12 = W2 W1, vv = W2 b1.  The host expands the
two-hop pairs (v, w) with merged path weights sum_u dis[u]^2, pre-scales
each token row xt[w] = dis[w] x[w] by its pair weight, and lays tokens
out in sub-tiles of <=32 destinations and <=128 tokens.  The device does
a single one-hot aggregation (3 matmuls of 32 cols per sub-tile, K~126)
straight into the projection layout [feat-in-block, dest], then projects
with W12 per 128-dest group (4 sub-tiles) and adds the rank-2 bias
correction via a K=2 matmul.  One aggregation pass, no intermediate
DRAM tensors: ~53MB HBM traffic and ~200k PE columns per core.
"""
import sys

sys.path.insert(0, "/opt/trn_rl_repo")

import numpy as np
import ml_dtypes

BF16 = ml_dtypes.bfloat16
FP8 = ml_dtypes.float8_e4m3

N_NODES = 100000
N_EDGES = 200000
H = 384
KB = H // 128
M_CORES = 8
NPC = N_NODES // M_CORES
WD = 32               # dests per sub-tile (one-hot width)
ST = 4                # sub-tiles per group (group = 128 dests)
SG = 4                # groups per output DMA
LWIN = 16             # sub-blocks per DMA window (= 4 groups)


def _pack2(sizes, captok=128, capcnt=WD):
    """Greedy one-bin-at-a-time pack of (nonzero-size) dests into
    sub-tiles with <=captok tokens and <=capcnt dests: fill each bin with
    the largest remaining size that fits.  Returns (sub_of, nb)."""
    import heapq
    order = np.argsort(-sizes, kind="stable")
    nb = max(int(np.ceil(sizes.sum() / captok)), 1)
    heap = [(0, 0, b) for b in range(nb)]
    heapq.heapify(heap)
    sub_of = np.empty(sizes.size, np.int64)
    for i in order:
        s = int(sizes[i])
        parked = []
        while True:
            if not heap:
                tok, cnt, b = 0, 0, nb
                nb += 1
                break
            tok, cnt, b = heapq.heappop(heap)
            if tok + s <= captok and cnt < capcnt:
                break
            parked.append((tok, cnt, b))
        for p in parked:
            heapq.heappush(heap, p)
        sub_of[i] = b
        tok += s
        cnt += 1
        if tok < captok and cnt < capcnt:
            heapq.heappush(heap, (tok, cnt, b))
    return sub_of, nb


def _prep(x, edge_index, W1, b1, W2, b2):
    row = np.asarray(edge_index[0], dtype=np.int64)
    col = np.asarray(edge_index[1], dtype=np.int64)
    xf = np.asarray(x, dtype=np.float64)

    deg = np.bincount(row, minlength=N_NODES).astype(np.float64)
    dis = deg ** -0.5
    a = np.bincount(col, weights=dis[row], minlength=N_NODES)
    cvec = dis * a
    c2 = dis * np.bincount(col, weights=(dis * cvec)[row], minlength=N_NODES)
    W12 = np.asarray(W2, np.float64) @ np.asarray(W1, np.float64)
    vv = np.asarray(W2, np.float64) @ np.asarray(b1, np.float64)
    xt = (dis[:, None] * xf).astype(np.float32)

    # two-hop pairs: for each edge e=(u,v), expand in-edges (w,u)
    indeg = np.bincount(col, minlength=N_NODES)
    order_c = np.argsort(col, kind="stable")
    row_by_col = row[order_c]
    coff = np.zeros(N_NODES + 1, np.int64)
    np.cumsum(indeg, out=coff[1:])

    cnts = indeg[row]
    P = int(cnts.sum())
    csum = np.zeros(N_EDGES + 1, np.int64)
    np.cumsum(cnts, out=csum[1:])
    eidx = np.repeat(np.arange(N_EDGES), cnts)
    rank = np.arange(P) - np.repeat(csum[:-1], cnts)
    w_pair = row_by_col[np.repeat(coff[row], cnts) + rank]
    v_pair = col[eidx]
    wt_pair = (dis[row] ** 2)[eidx]
    key = v_pair * N_NODES + w_pair
    ukey, inv = np.unique(key, return_inverse=True)
    wts = np.bincount(inv, weights=wt_pair)
    v_m = ukey // N_NODES
    w_m = ukey % N_NODES

    w12 = np.ascontiguousarray(
        W12.T.astype(BF16).reshape(KB, 128, H).transpose(1, 0, 2))
    vb2 = np.stack([vv.astype(BF16),
                    np.asarray(b2, np.float64).astype(BF16)], axis=0)

    # per-core packing (shared program => shared NG)
    cores = []
    for cc in range(M_CORES):
        m = (v_m // NPC) == cc
        dlo = v_m[m] - cc * NPC
        wsrc = w_m[m]
        wt = wts[m]
        sizes = np.bincount(dlo, minlength=NPC)
        # pack nonzero dests token-tight, then fill count slots with zeros
        nz = np.nonzero(sizes > 0)[0]
        sub_nz, nb = _pack2(sizes[nz])
        sub_of = np.empty(NPC, np.int64)
        sub_of[nz] = sub_nz
        zr = np.nonzero(sizes == 0)[0]
        free = WD - np.bincount(sub_nz, minlength=nb)
        slots = np.repeat(np.arange(nb), free)
        if slots.size < zr.size:
            extra = -(-(zr.size - slots.size) // WD)
            slots = np.concatenate(
                [slots, np.repeat(np.arange(nb, nb + extra), WD)])
            nb += extra
        sub_of[zr] = slots[:zr.size]
        cores.append((dlo, wsrc, wt, sizes, sub_of, nb))
    NG = max(-(-c[5] // ST) for c in cores)
    NG += (-NG) % (LWIN // ST)
    TOT = NG * ST

    in_maps = []
    perms = []
    for cc, (dlo, wsrc, wt, sizes, sub_of, nb) in enumerate(cores):
        # dest slot + token offset within sub-tile
        order = np.lexsort((np.arange(NPC), sub_of))
        dslot = np.empty(NPC, np.int64)
        tok_off = np.empty(NPC, np.int64)
        cnt = np.zeros(nb, np.int64)
        tok = np.zeros(nb, np.int64)
        for i in order:
            t = sub_of[i]
            dslot[i] = cnt[t]
            tok_off[i] = tok[t]
            cnt[t] += 1
            tok[t] += sizes[i]

        # pairs sorted by (dlo, w) already (ukey order)
        starts = np.zeros(NPC + 1, np.int64)
        np.cumsum(sizes, out=starts[1:])
        prank = np.arange(dlo.size) - starts[dlo]
        part = tok_off[dlo] + prank
        sb = sub_of[dlo]

        t2 = np.zeros((128, TOT, H), dtype=BF16)
        t2[part, sb] = (xt[wsrc] * wt[:, None].astype(np.float32))
        s1 = np.zeros((128, TOT, WD), dtype=FP8)
        s1[part, sb, dslot[dlo]] = 1.0

        g_d = sub_of // ST
        part_d = (sub_of % ST) * WD + dslot
        dd = dis[cc * NPC:(cc + 1) * NPC]
        disc = np.zeros((128, NG), dtype=np.float32)
        disc[part_d, g_d] = dd.astype(np.float32)

        in_maps.append({
            "t2": t2, "s1": s1, "disc": disc, "w12": w12,
        })
        perms.append(part_d * NG + g_d)
    # x-independent rank-2 bias, added host-side after the gather
    corr = (np.outer(c2, vv) + np.outer(cvec, np.asarray(b2, np.float64))
            ).astype(np.float32)
    return in_maps, dict(NG=NG), perms, corr


def _build(dims):
    import concourse.bass as bass
    import concourse.bacc as bacc
    import concourse.mybir as mybir
    import concourse.tile as tile

    dt = mybir.dt
    AF = mybir.ActivationFunctionType
    NG = dims["NG"]
    TOT = NG * ST
    NW = TOT // LWIN

    nc = bacc.Bacc(None, target_bir_lowering=False)
    t2 = nc.dram_tensor("t2", [128, TOT, H], dt.bfloat16, kind="ExternalInput")
    s1 = nc.dram_tensor("s1", [128, TOT, WD], dt.float8e4, kind="ExternalInput")
    disc = nc.dram_tensor("disc", [128, NG], dt.float32, kind="ExternalInput")
    w12 = nc.dram_tensor("w12", [128, KB, H], dt.bfloat16, kind="ExternalInput")
    out = nc.dram_tensor("out", [128, NG, H], dt.bfloat16, kind="ExternalOutput")

    with tile.TileContext(nc) as tc:
        with (
            tc.tile_pool(name="const", bufs=1) as cp,
            tc.tile_pool(name="io", bufs=3) as iop,
            tc.tile_pool(name="stg", bufs=2) as stgp,
            tc.tile_pool(name="ps", bufs=2, space="PSUM") as psp,
        ):
            wins = {}
            HW = LWIN // 2

            def load_window(nx):
                tt = iop.tile([128, LWIN, H], dt.bfloat16, tag="t2", bufs=5)
                nc.sync.dma_start(tt[:, :HW],
                                  t2[:, nx * LWIN:nx * LWIN + HW, :])
                nc.scalar.dma_start(tt[:, HW:],
                                    t2[:, nx * LWIN + HW:(nx + 1) * LWIN, :])
                ss = iop.tile([128, LWIN, WD], dt.float8e4, tag="s1", bufs=5)
                nc.sync.dma_start(ss[:, :HW], s1[:, nx * LWIN:nx * LWIN + HW, :])
                nc.scalar.dma_start(ss[:, HW:],
                                    s1[:, nx * LWIN + HW:(nx + 1) * LWIN, :])
                wins[nx] = (tt, ss)
                wins.pop(nx - 5, None)
                ensure.hi = nx

            def ensure(wi):
                while ensure.hi < min(wi + 2, NW - 1) or ensure.hi < wi:
                    load_window(ensure.hi + 1)
            ensure.hi = -1

            # window 0, ordered so group 0 can start ASAP: tiny s1 first,
            # then sub-block-granular t2 for the first group, consts after
            tt0 = iop.tile([128, LWIN, H], dt.bfloat16, tag="t2", bufs=5)
            ss0 = iop.tile([128, LWIN, WD], dt.float8e4, tag="s1", bufs=5)
            nc.sync.dma_start(ss0[:, :HW], s1[:, :HW, :])
            nc.scalar.dma_start(ss0[:, HW:], s1[:, HW:LWIN, :])
            for sb0 in range(ST):
                nc.sync.dma_start(tt0[:, sb0:sb0 + 1], t2[:, sb0:sb0 + 1, :])
            w12_sb = cp.tile([128, KB, H], dt.bfloat16)
            nc.sync.dma_start(w12_sb[:], w12[:])
            disc_sb = cp.tile([128, NG], dt.float32)
            nc.scalar.dma_start(disc_sb[:], disc[:])
            nc.sync.dma_start(tt0[:, ST:HW], t2[:, ST:HW, :])
            nc.scalar.dma_start(tt0[:, HW:], t2[:, HW:LWIN, :])
            wins[0] = (tt0, ss0)
            ensure.hi = 0

            def emit_psC(j):
                wi = (j * ST) // LWIN
                ensure(wi)
                tt, ss = wins[wi]
                psC = psp.tile([128, H], dt.float32, tag="psC", bufs=4)
                for st in range(ST):
                    sb = (j * ST) % LWIN + st
                    for f in range(KB):
                        o = f * 128 + st * WD
                        nc.tensor.matmul(psC[:, o:o + WD],
                                         tt[:, sb, f * 128:(f + 1) * 128],
                                         ss[:, sb, :], start=True, stop=True)
                zf = iop.tile([128, H], dt.bfloat16, tag="zf", bufs=4)
                if j % 2 == 0:
                    nc.vector.tensor_copy(zf[:], psC[:])
                else:
                    nc.scalar.activation(zf[:], psC[:], AF.Copy)
                return zf

            def emit_psD(j, zf, ows):
                psD = psp.tile([128, H], dt.float32, tag="psD", bufs=3)
                for k in range(KB):
                    nc.tensor.matmul(psD[:], zf[:, k * 128:(k + 1) * 128],
                                     w12_sb[:, k, :], start=(k == 0),
                                     stop=(k == KB - 1))
                if j % 2 == 0:
                    nc.scalar.activation(ows[:, j % SG, :], psD[:], AF.Copy,
                                         scale=disc_sb[:, j:j + 1])
                else:
                    nc.vector.tensor_scalar_mul(ows[:, j % SG, :], psD[:],
                                                disc_sb[:, j:j + 1])

            from collections import deque
            pend = deque()
            ows = None
            for j in range(NG + 2):
                if j < NG:
                    pend.append((j, emit_psC(j)))
                if len(pend) > 2 or (j >= NG and pend):
                    pj, pzf = pend.popleft()
                    if pj % SG == 0:
                        ows = stgp.tile([128, SG, H], dt.bfloat16, tag="ows",
                                        bufs=4)
                    emit_psD(pj, pzf, ows)
                    if pj % SG == SG - 1:
                        if (pj // SG) >= NG // SG - 4:
                            eng = nc.sync if (pj // SG) % 2 == 0 else nc.scalar
                        else:
                            eng = nc.gpsimd
                        eng.dma_start(out[:, pj - SG + 1:pj + 1, :], ows[:])
    nc.compile()
    return nc


_CACHE = {}


def kernel(x, edge_index, W1, b1, W2, b2):
    from concourse import bass_utils

    in_maps, dims, perms, corr = _prep(x, edge_index, W1, b1, W2, b2)
    key = dims["NG"]
    if key not in _CACHE:
        _CACHE[key] = _build(dims)
    nc = _CACHE[key]
    res = bass_utils.run_bass_kernel_spmd(nc, in_maps, core_ids=list(range(M_CORES)))
    NG = dims["NG"]
    out = np.empty((N_NODES, H), np.float32)
    for cc in range(M_CORES):
        flat = np.asarray(res.results[cc]["out"]).reshape(128 * NG, H)
        out[cc * NPC:(cc + 1) * NPC] = flat[perms[cc]].astype(np.float32)
    out += corr
    return out


# revision 27
# speedup vs baseline: 1.0559x; 1.0559x over previous
"""GCN 2-layer message-passing block on 8 Trainium2 NeuronCores — v4.

Collapsed algebra: z = (S^2 x) W12^T + c2 vv^T + cvec b2^T with
S = D^-1/2 A D^-1/2, W12 = W2 W1, vv = W2 b1.  The host expands the
two-hop pairs (v, w) with merged path weights sum_u dis[u]^2, pre-scales
each token row xt[w] = dis[w] x[w] by its pair weight, and lays tokens
out in sub-tiles of <=32 destinations and <=128 tokens.  The device does
a single one-hot aggregation (3 matmuls of 32 cols per sub-tile, K~126)
straight into the projection layout [feat-in-block, dest], then projects
with W12 per 128-dest group (4 sub-tiles) and adds the rank-2 bias
correction via a K=2 matmul.  One aggregation pass, no intermediate
DRAM tensors: ~53MB HBM traffic and ~200k PE columns per core.
"""
import sys

sys.path.insert(0, "/opt/trn_rl_repo")

import numpy as np
import ml_dtypes

BF16 = ml_dtypes.bfloat16
FP8 = ml_dtypes.float8_e4m3

N_NODES = 100000
N_EDGES = 200000
H = 384
KB = H // 128
M_CORES = 8
NPC = N_NODES // M_CORES
WD = 32               # dests per sub-tile (one-hot width)
ST = 4                # sub-tiles per group (group = 128 dests)
SG = 4                # groups per output DMA
LWIN = 16             # sub-blocks per DMA window (= 4 groups)


def _pack2(sizes, captok=128, capcnt=WD):
    """Greedy one-bin-at-a-time pack of (nonzero-size) dests into
    sub-tiles with <=captok tokens and <=capcnt dests: fill each bin with
    the largest remaining size that fits.  Returns (sub_of, nb)."""
    import heapq
    order = np.argsort(-sizes, kind="stable")
    nb = max(int(np.ceil(sizes.sum() / captok)), 1)
    heap = [(0, 0, b) for b in range(nb)]
    heapq.heapify(heap)
    sub_of = np.empty(sizes.size, np.int64)
    for i in order:
        s = int(sizes[i])
        parked = []
        while True:
            if not heap:
                tok, cnt, b = 0, 0, nb
                nb += 1
                break
            tok, cnt, b = heapq.heappop(heap)
            if tok + s <= captok and cnt < capcnt:
                break
            parked.append((tok, cnt, b))
        for p in parked:
            heapq.heappush(heap, p)
        sub_of[i] = b
        tok += s
        cnt += 1
        if tok < captok and cnt < capcnt:
            heapq.heappush(heap, (tok, cnt, b))
    return sub_of, nb


def _prep(x, edge_index, W1, b1, W2, b2):
    row = np.asarray(edge_index[0], dtype=np.int64)
    col = np.asarray(edge_index[1], dtype=np.int64)
    xf = np.asarray(x, dtype=np.float64)

    deg = np.bincount(row, minlength=N_NODES).astype(np.float64)
    dis = deg ** -0.5
    a = np.bincount(col, weights=dis[row], minlength=N_NODES)
    cvec = dis * a
    c2 = dis * np.bincount(col, weights=(dis * cvec)[row], minlength=N_NODES)
    W12 = np.asarray(W2, np.float64) @ np.asarray(W1, np.float64)
    vv = np.asarray(W2, np.float64) @ np.asarray(b1, np.float64)
    xt = (dis[:, None] * xf).astype(np.float32)

    # two-hop pairs: for each edge e=(u,v), expand in-edges (w,u)
    indeg = np.bincount(col, minlength=N_NODES)
    order_c = np.argsort(col, kind="stable")
    row_by_col = row[order_c]
    coff = np.zeros(N_NODES + 1, np.int64)
    np.cumsum(indeg, out=coff[1:])

    cnts = indeg[row]
    P = int(cnts.sum())
    csum = np.zeros(N_EDGES + 1, np.int64)
    np.cumsum(cnts, out=csum[1:])
    eidx = np.repeat(np.arange(N_EDGES), cnts)
    rank = np.arange(P) - np.repeat(csum[:-1], cnts)
    w_pair = row_by_col[np.repeat(coff[row], cnts) + rank]
    v_pair = col[eidx]
    wt_pair = (dis[row] ** 2)[eidx]
    key = v_pair * N_NODES + w_pair
    ukey, inv = np.unique(key, return_inverse=True)
    wts = np.bincount(inv, weights=wt_pair)
    v_m = ukey // N_NODES
    w_m = ukey % N_NODES

    w12 = np.ascontiguousarray(
        W12.T.astype(BF16).reshape(KB, 128, H).transpose(1, 0, 2))
    vb2 = np.stack([vv.astype(BF16),
                    np.asarray(b2, np.float64).astype(BF16)], axis=0)

    # per-core packing (shared program => shared NG)
    cores = []
    for cc in range(M_CORES):
        m = (v_m // NPC) == cc
        dlo = v_m[m] - cc * NPC
        wsrc = w_m[m]
        wt = wts[m]
        sizes = np.bincount(dlo, minlength=NPC)
        # pack nonzero dests token-tight, then fill count slots with zeros
        nz = np.nonzero(sizes > 0)[0]
        sub_nz, nb = _pack2(sizes[nz])
        sub_of = np.empty(NPC, np.int64)
        sub_of[nz] = sub_nz
        zr = np.nonzero(sizes == 0)[0]
        free = WD - np.bincount(sub_nz, minlength=nb)
        slots = np.repeat(np.arange(nb), free)
        if slots.size < zr.size:
            extra = -(-(zr.size - slots.size) // WD)
            slots = np.concatenate(
                [slots, np.repeat(np.arange(nb, nb + extra), WD)])
            nb += extra
        sub_of[zr] = slots[:zr.size]
        cores.append((dlo, wsrc, wt, sizes, sub_of, nb))
    NG = max(-(-c[5] // ST) for c in cores)
    NG += (-NG) % (LWIN // ST)
    TOT = NG * ST

    in_maps = []
    perms = []
    for cc, (dlo, wsrc, wt, sizes, sub_of, nb) in enumerate(cores):
        # dest slot + token offset within sub-tile
        order = np.lexsort((np.arange(NPC), sub_of))
        dslot = np.empty(NPC, np.int64)
        tok_off = np.empty(NPC, np.int64)
        cnt = np.zeros(nb, np.int64)
        tok = np.zeros(nb, np.int64)
        for i in order:
            t = sub_of[i]
            dslot[i] = cnt[t]
            tok_off[i] = tok[t]
            cnt[t] += 1
            tok[t] += sizes[i]

        # pairs sorted by (dlo, w) already (ukey order)
        starts = np.zeros(NPC + 1, np.int64)
        np.cumsum(sizes, out=starts[1:])
        prank = np.arange(dlo.size) - starts[dlo]
        part = tok_off[dlo] + prank
        sb = sub_of[dlo]

        t2 = np.zeros((128, TOT, H), dtype=BF16)
        t2[part, sb] = (xt[wsrc] * wt[:, None].astype(np.float32))
        s1 = np.zeros((128, TOT, WD), dtype=FP8)
        s1[part, sb, dslot[dlo]] = 1.0

        g_d = sub_of // ST
        part_d = (sub_of % ST) * WD + dslot
        dd = dis[cc * NPC:(cc + 1) * NPC]
        disc = np.zeros((128, NG), dtype=np.float32)
        disc[part_d, g_d] = dd.astype(np.float32)

        in_maps.append({
            "t2": t2, "s1": s1, "disc": disc, "w12": w12,
        })
        perms.append(part_d * NG + g_d)
    # x-independent rank-2 bias, added host-side after the gather
    corr = (np.outer(c2, vv) + np.outer(cvec, np.asarray(b2, np.float64))
            ).astype(np.float32)
    return in_maps, dict(NG=NG), perms, corr


def _build(dims):
    import concourse.bass as bass
    import concourse.bacc as bacc
    import concourse.mybir as mybir
    import concourse.tile as tile

    dt = mybir.dt
    AF = mybir.ActivationFunctionType
    NG = dims["NG"]
    TOT = NG * ST
    NW = TOT // LWIN

    nc = bacc.Bacc(None, target_bir_lowering=False)
    t2 = nc.dram_tensor("t2", [128, TOT, H], dt.bfloat16, kind="ExternalInput")
    s1 = nc.dram_tensor("s1", [128, TOT, WD], dt.float8e4, kind="ExternalInput")
    disc = nc.dram_tensor("disc", [128, NG], dt.float32, kind="ExternalInput")
    w12 = nc.dram_tensor("w12", [128, KB, H], dt.bfloat16, kind="ExternalInput")
    out = nc.dram_tensor("out", [128, NG, H], dt.bfloat16, kind="ExternalOutput")

    with tile.TileContext(nc) as tc:
        with (
            tc.tile_pool(name="const", bufs=1) as cp,
            tc.tile_pool(name="io", bufs=3) as iop,
            tc.tile_pool(name="stg", bufs=2) as stgp,
            tc.tile_pool(name="ps", bufs=2, space="PSUM") as psp,
        ):
            wins = {}
            HW = LWIN // 2

            def load_window(nx):
                tt = iop.tile([128, LWIN, H], dt.bfloat16, tag="t2", bufs=7)
                nc.sync.dma_start(tt[:, :HW],
                                  t2[:, nx * LWIN:nx * LWIN + HW, :])
                nc.scalar.dma_start(tt[:, HW:],
                                    t2[:, nx * LWIN + HW:(nx + 1) * LWIN, :])
                ss = iop.tile([128, LWIN, WD], dt.float8e4, tag="s1", bufs=7)
                nc.sync.dma_start(ss[:, :HW], s1[:, nx * LWIN:nx * LWIN + HW, :])
                nc.scalar.dma_start(ss[:, HW:],
                                    s1[:, nx * LWIN + HW:(nx + 1) * LWIN, :])
                wins[nx] = (tt, ss)
                wins.pop(nx - 7, None)
                ensure.hi = nx

            def ensure(wi):
                while ensure.hi < min(wi + 3, NW - 1) or ensure.hi < wi:
                    load_window(ensure.hi + 1)
            ensure.hi = -1

            # window 0, ordered so group 0 can start ASAP: tiny s1 first,
            # then sub-block-granular t2 for the first group, consts after
            tt0 = iop.tile([128, LWIN, H], dt.bfloat16, tag="t2", bufs=7)
            ss0 = iop.tile([128, LWIN, WD], dt.float8e4, tag="s1", bufs=7)
            nc.sync.dma_start(ss0[:, :HW], s1[:, :HW, :])
            nc.scalar.dma_start(ss0[:, HW:], s1[:, HW:LWIN, :])
            for sb0 in range(ST):
                nc.sync.dma_start(tt0[:, sb0:sb0 + 1], t2[:, sb0:sb0 + 1, :])
            w12_sb = cp.tile([128, KB, H], dt.bfloat16)
            nc.sync.dma_start(w12_sb[:], w12[:])
            disc_sb = cp.tile([128, NG], dt.float32)
            nc.scalar.dma_start(disc_sb[:], disc[:])
            nc.sync.dma_start(tt0[:, ST:HW], t2[:, ST:HW, :])
            nc.scalar.dma_start(tt0[:, HW:], t2[:, HW:LWIN, :])
            wins[0] = (tt0, ss0)
            ensure.hi = 0

            def emit_psC(j):
                wi = (j * ST) // LWIN
                ensure(wi)
                tt, ss = wins[wi]
                psC = psp.tile([128, H], dt.float32, tag="psC", bufs=4)
                for st in range(ST):
                    sb = (j * ST) % LWIN + st
                    for f in range(KB):
                        o = f * 128 + st * WD
                        nc.tensor.matmul(psC[:, o:o + WD],
                                         tt[:, sb, f * 128:(f + 1) * 128],
                                         ss[:, sb, :], start=True, stop=True)
                zf = iop.tile([128, H], dt.bfloat16, tag="zf", bufs=4)
                if j % 2 == 0:
                    nc.vector.tensor_copy(zf[:], psC[:])
                else:
                    nc.scalar.activation(zf[:], psC[:], AF.Copy)
                return zf

            def emit_psD(j, zf, ows):
                psD = psp.tile([128, H], dt.float32, tag="psD", bufs=3)
                for k in range(KB):
                    nc.tensor.matmul(psD[:], zf[:, k * 128:(k + 1) * 128],
                                     w12_sb[:, k, :], start=(k == 0),
                                     stop=(k == KB - 1))
                if j % 2 == 0:
                    nc.scalar.activation(ows[:, j % SG, :], psD[:], AF.Copy,
                                         scale=disc_sb[:, j:j + 1])
                else:
                    nc.vector.tensor_scalar_mul(ows[:, j % SG, :], psD[:],
                                                disc_sb[:, j:j + 1])

            from collections import deque
            pend = deque()
            ows = None
            for j in range(NG + 2):
                if j < NG:
                    pend.append((j, emit_psC(j)))
                if len(pend) > 2 or (j >= NG and pend):
                    pj, pzf = pend.popleft()
                    if pj % SG == 0:
                        ows = stgp.tile([128, SG, H], dt.bfloat16, tag="ows",
                                        bufs=6)
                    emit_psD(pj, pzf, ows)
                    if pj % SG == SG - 1:
                        if (pj // SG) >= NG // SG - 4:
                            eng = nc.sync if (pj // SG) % 2 == 0 else nc.scalar
                        else:
                            eng = nc.gpsimd
                        eng.dma_start(out[:, pj - SG + 1:pj + 1, :], ows[:])
    nc.compile()
    return nc


_CACHE = {}


def kernel(x, edge_index, W1, b1, W2, b2):
    from concourse import bass_utils

    in_maps, dims, perms, corr = _prep(x, edge_index, W1, b1, W2, b2)
    key = dims["NG"]
    if key not in _CACHE:
        _CACHE[key] = _build(dims)
    nc = _CACHE[key]
    res = bass_utils.run_bass_kernel_spmd(nc, in_maps, core_ids=list(range(M_CORES)))
    NG = dims["NG"]
    out = np.empty((N_NODES, H), np.float32)
    for cc in range(M_CORES):
        flat = np.asarray(res.results[cc]["out"]).reshape(128 * NG, H)
        out[cc * NPC:(cc + 1) * NPC] = flat[perms[cc]].astype(np.float32)
    out += corr
    return out
